# revision 10
# baseline (speedup 1.0000x reference)
"""3-layer GAT (graph attention network) on Trainium2 — Bass/Tile, 8-core SPMD.

Sharding: nodes are partitioned into 8 contiguous ranges (graph/data
parallel).  Each core owns the edges whose *destination* falls in its range.
Per layer:
  phase A : feat = x @ W for the core's own node slice (PE), attention logits
            el = feat.al and er = feat.ar computed as x @ (W@al_flat) on PE.
            A combined per-node table row [feat(bf16) | el(bf16) | pad] is
            written so ONE gather fetches both.
  AllGather the combined table so every core can gather arbitrary src rows.
  edge    : bulk dma_gather of table[src] (el comes along free) and of
            er[dst]; w = exp(leaky_relu(el+er)); segment-sum of w*feat and w
            over destination nodes via PE matmuls with on-device-built
            one-hot matrices accumulated in PSUM; epilogue divides by the
            summed w (edge softmax), adds bias, applies relu.
Edge softmax skips the segment-max subtraction: alpha = exp(e)/sum(exp(e))
is mathematically identical and the logits here are O(1).

dma_gather uses int16 indices (max 32767), so edges are split per PSUM group
into a "lo" zone (src < 32768) and a "hi" zone (src >= 32768, gathered from a
base-offset view of the table).  All indices are valid (pads point at row 0
and are killed by a -1 dstrel -> all-zero one-hot column), so descriptor
counts are compile-time constants — one SPMD program serves all 8 cores; all
data-dependent structure lives in host-built index tables.
"""

import numpy as np

try:
    import ml_dtypes
    _BF16 = ml_dtypes.bfloat16
except ImportError:  # pragma: no cover
    _BF16 = None

# ---------------- static problem config (self-contained) ---------------------
N_CORES = 8
NEG_SLOPE = 0.2
P = 128
GROUP_E = 1024             # max edges per PSUM accumulation group
CHUNK_GROUPS = 4           # groups per gather chunk
SPLIT = 32768              # int16 index split point
# (in_dim, H, D, apply_relu) per layer
LAYERS = [(128, 4, 32, True), (128, 4, 32, True), (128, 1, 64, False)]
OUT_DIM = 64
ROWS_L = (256, 256, 128)   # combined table row per layer, bf16 elems
EL_OFF = None              # el lives right after feat (offset = hd)

_cache = {}
last_run_info = {}


# ============================ host-side preprocessing ========================

def _wrap16(vals, cols):
    """dma_gather index layout: entry i -> [i % 16, i // 16], replicated
    across the 8 groups of 16 partitions."""
    t = np.zeros((16, cols), np.int16)
    n = len(vals)
    t[np.arange(n) % 16, np.arange(n) // 16] = vals.astype(np.int16)
    return np.tile(t, (8, 1))


def _preprocess(src, dst, n_nodes, n_cores):
    npc = n_nodes // n_cores
    xj = (npc + P - 1) // P
    cores = []
    for c in range(n_cores):
        lo = c * npc
        m = (dst >= lo) & (dst < lo + npc)
        s = src[m].astype(np.int64)
        d = (dst[m] - lo).astype(np.int64)
        o = np.argsort(d, kind="stable")
        s, d = s[o], d[o]
        counts = np.bincount(d, minlength=npc)
        cum = np.zeros(npc + 1, np.int64)
        np.cumsum(counts, out=cum[1:])
        groups = []
        base = 0
        while base < npc:
            dmax = min(base + P, npc)
            limit = cum[base] + GROUP_E
            dend = int(np.searchsorted(cum, limit, side="right")) - 1
            dend = min(dend, dmax)
            if dend <= base:
                raise ValueError(f"dst {base} has degree > {GROUP_E}")
            groups.append((base, int(cum[base]), int(cum[dend])))
            base = dend
        cores.append((s, d, groups))

    ng = max(len(g) for (_, _, g) in cores)
    ng = ((ng + CHUNK_GROUPS - 1) // CHUNK_GROUPS) * CHUNK_GROUPS
    # per-group-index lo/hi tile counts (max across cores -> shared program)
    TL = np.zeros(ng, np.int64)
    TH = np.zeros(ng, np.int64)
    for (s, d, groups) in cores:
        for gi, (b, e0, e1) in enumerate(groups):
            nlo = int((s[e0:e1] < SPLIT).sum())
            nhi = (e1 - e0) - nlo
            TL[gi] = max(TL[gi], (nlo + P - 1) // P)
            TH[gi] = max(TH[gi], (nhi + P - 1) // P)
    lo_base = np.zeros(ng + 1, np.int64)   # tile offsets within lo zone
    hi_base = np.zeros(ng + 1, np.int64)
    np.cumsum(TL, out=lo_base[1:])
    np.cumsum(TH, out=hi_base[1:])
    SL = int(lo_base[ng]) * P              # lo-zone slots
    SH = int(hi_base[ng]) * P
    SLP = max(SL, 2048)                    # padded sizes for tensor shapes
    SHP = max(SH, 2048)

    per_core = []
    for c, (s, d, groups) in enumerate(cores):
        idx_lo = np.zeros(SL, np.int64)
        idx_hi = np.zeros(SH, np.int64)
        er_lo = np.zeros(SL, np.int64)
        er_hi = np.zeros(SH, np.int64)
        dr_lo = np.full(SL, -1.0, np.float32)
        dr_hi = np.full(SH, -1.0, np.float32)
        for gi, (b, e0, e1) in enumerate(groups):
            eg_s = s[e0:e1]
            eg_d = d[e0:e1]
            lm = eg_s < SPLIT
            for zone, msk in ((0, lm), (1, ~lm)):
                zs = eg_s[msk]
                zd = eg_d[msk]
                n = len(zs)
                o = (int(lo_base[gi]) if zone == 0 else int(hi_base[gi])) * P
                tgt_i = idx_lo if zone == 0 else idx_hi
                tgt_e = er_lo if zone == 0 else er_hi
                tgt_r = dr_lo if zone == 0 else dr_hi
                tgt_i[o:o + n] = zs - (0 if zone == 0 else SPLIT)
                tgt_e[o:o + n] = zd
                tgt_r[o:o + n] = (zd - b).astype(np.float32)
        # scratch-row map
        srow = np.zeros(npc, np.int64)
        for gi, (b, e0, e1) in enumerate(groups):
            b_next = groups[gi + 1][0] if gi + 1 < len(groups) else npc
            srow[b:b_next] = gi * P + (np.arange(b, b_next) - b)
        nn = np.arange(xj * P)
        xi = np.zeros(xj * P, np.int64)
        valid = nn < npc
        xi[valid] = srow[nn[valid]]
        def _padcols(a, cols):
            out = np.zeros((a.shape[0], cols), a.dtype)
            out[:, :a.shape[1]] = a
            return out

        dl = dr_lo.reshape(SL // P, P).T.astype(_BF16) if SL else \
            np.zeros((P, 0), _BF16)
        dh = dr_hi.reshape(SH // P, P).T.astype(_BF16) if SH else \
            np.zeros((P, 0), _BF16)
        per_core.append(dict(
            idx_lo=_padcols(_wrap16(idx_lo, max(SL // 16, 1)), SLP // 16),
            idx_hi=_padcols(_wrap16(idx_hi, max(SH // 16, 1)), SHP // 16),
            idx_er_lo=_padcols(_wrap16(er_lo, max(SL // 16, 1)), SLP // 16),
            idx_er_hi=_padcols(_wrap16(er_hi, max(SH // 16, 1)), SHP // 16),
            dr_lo=_padcols(dl, SLP // P),
            dr_hi=_padcols(dh, SHP // P),
            x_idx=_wrap16(xi, (xj * P) // 16),
            srow=srow,
        ))
    meta = dict(ng=ng, TL=tuple(int(x) for x in TL),
                TH=tuple(int(x) for x in TH), SL=SL, SH=SH,
                SLP=SLP, SHP=SHP,
                xj=xj, npc=npc, n_nodes=n_nodes, n_cores=n_cores)
    return meta, per_core


# ============================ device program =================================

def _build_program(meta):
    import concourse.bass as bass
    import concourse.tile as tile
    from concourse import bacc, mybir

    def _midb(ap, n):
        # [P, D] -> [P, n, D] with the middle dim broadcast (step 0)
        return bass.AP(ap.tensor, ap.offset,
                       [list(ap.ap[0]), [0, n], list(ap.ap[1])])

    f32 = mybir.dt.float32
    bf16 = mybir.dt.bfloat16
    i16 = mybir.dt.int16
    AF = mybir.ActivationFunctionType
    OP = mybir.AluOpType

    ng, SL, SH = meta["ng"], meta["SL"], meta["SH"]
    SLP, SHP = meta["SLP"], meta["SHP"]
    TL, TH = meta["TL"], meta["TH"]
    xj, npc = meta["xj"], meta["npc"]
    n_nodes, n_cores = meta["n_nodes"], meta["n_cores"]
    NPCP = xj * P
    lo_base = np.concatenate([[0], np.cumsum(TL)]).astype(int)
    hi_base = np.concatenate([[0], np.cumsum(TH)]).astype(int)
    nchunk = ng // CHUNK_GROUPS

    nc = bacc.Bacc("TRN2", target_bir_lowering=False, debug=False,
                   enable_asserts=False, num_devices=n_cores)

    t_feats = nc.dram_tensor("features_own", [NPCP, 128], f32,
                             kind="ExternalInput").ap()
    t_idx_lo = nc.dram_tensor("idx_lo", [P, SLP // 16], i16,
                              kind="ExternalInput").ap()
    t_idx_hi = nc.dram_tensor("idx_hi", [P, SHP // 16], i16,
                              kind="ExternalInput").ap()
    t_ier_lo = nc.dram_tensor("idx_er_lo", [P, SLP // 16], i16,
                              kind="ExternalInput").ap()
    t_ier_hi = nc.dram_tensor("idx_er_hi", [P, SHP // 16], i16,
                              kind="ExternalInput").ap()
    t_dr_lo = nc.dram_tensor("dr_lo", [P, SLP // P], bf16,
                             kind="ExternalInput").ap()
    t_dr_hi = nc.dram_tensor("dr_hi", [P, SHP // P], bf16,
                             kind="ExternalInput").ap()
    t_x_idx = nc.dram_tensor("x_idx", [P, NPCP // 16], i16,
                             kind="ExternalInput").ap()
    t_iota = nc.dram_tensor("iota_rep", [P, P], bf16,
                            kind="ExternalInput").ap()
    t_ident = nc.dram_tensor("identity", [P, P], f32,
                             kind="ExternalInput").ap()
    t_W, t_Wal, t_War, t_b = [], [], [], []
    for li, (ind, H, D, _) in enumerate(LAYERS):
        hd = H * D
        t_W.append(nc.dram_tensor(f"W{li}", [ind, hd], f32,
                                  kind="ExternalInput").ap())
        t_Wal.append(nc.dram_tensor(f"Wal{li}", [ind, H], f32,
                                    kind="ExternalInput").ap())
        t_War.append(nc.dram_tensor(f"War{li}", [ind, H], f32,
                                    kind="ExternalInput").ap())
        t_b.append(nc.dram_tensor(f"br{li}", [P, hd], f32,
                                  kind="ExternalInput").ap())
    t_out = nc.dram_tensor("out", [ng * P, OUT_DIM], f32,
                           kind="ExternalOutput").ap()

    with tile.TileContext(nc) as tc:
        with (
            tc.tile_pool(name="const", bufs=1) as cpool,
            tc.tile_pool(name="big", bufs=1) as bigpool,
            tc.tile_pool(name="sb", bufs=3) as sb,
            tc.tile_pool(name="fg", bufs=2) as fgpool,
            tc.tile_pool(name="wp", bufs=2) as wpool,
            tc.tile_pool(name="ps", bufs=3, space="PSUM") as pspool,
            tc.tile_pool(name="psd", bufs=3, space="PSUM") as psdpool,
            tc.tile_pool(name="psA", bufs=2, space="PSUM") as psA,
            tc.tile_pool(name="dram", bufs=1, space="DRAM") as dram,
        ):
            # ---- constants ----
            ident = cpool.tile([P, P], f32)
            nc.sync.dma_start(ident[:], t_ident)
            iota = cpool.tile([P, P], bf16)
            nc.sync.dma_start(iota[:], t_iota)
            idx_lo = cpool.tile([P, SLP // 16], i16)
            nc.sync.dma_start(idx_lo[:], t_idx_lo)
            idx_hi = cpool.tile([P, SHP // 16], i16)
            nc.sync.dma_start(idx_hi[:], t_idx_hi)
            ier_lo = cpool.tile([P, SLP // 16], i16)
            nc.sync.dma_start(ier_lo[:], t_ier_lo)
            ier_hi = cpool.tile([P, SHP // 16], i16)
            nc.sync.dma_start(ier_hi[:], t_ier_hi)
            dr_lo = cpool.tile([P, SLP // P], bf16)
            nc.sync.dma_start(dr_lo[:], t_dr_lo)
            dr_hi = cpool.tile([P, SHP // P], bf16)
            nc.sync.dma_start(dr_hi[:], t_dr_hi)
            xidx = cpool.tile([P, NPCP // 16], i16)
            nc.sync.dma_start(xidx[:], t_x_idx)
            Ws, Wals, Wars, Bs = [], [], [], []
            for li, (ind, H, D, _) in enumerate(LAYERS):
                hd = H * D
                w = cpool.tile([ind, hd], f32, tag=f"W{li}")
                nc.sync.dma_start(w[:], t_W[li])
                Ws.append(w)
                wa = cpool.tile([ind, H], f32, tag=f"Wal{li}")
                nc.sync.dma_start(wa[:], t_Wal[li])
                Wals.append(wa)
                wr = cpool.tile([ind, H], f32, tag=f"War{li}")
                nc.sync.dma_start(wr[:], t_War[li])
                Wars.append(wr)
                bb = cpool.tile([P, hd], f32, tag=f"br{li}")
                nc.sync.dma_start(bb[:], t_b[li])
                Bs.append(bb)

            prev_scratch = None
            for li, (ind, H, D, apply_relu) in enumerate(LAYERS):
                hd = H * D
                # ---------------- phase A ----------------
                x_own = bigpool.tile([P, xj * ind], f32, tag="x_own")
                if li == 0:
                    nc.sync.dma_start(
                        x_own[:].rearrange("p (i d) -> p i d", d=ind),
                        t_feats.rearrange("(i p) d -> p i d", p=P))
                else:
                    nc.gpsimd.dma_gather(
                        out_ap=x_own[:].rearrange("p (i d) -> p i d", d=ind),
                        in_ap=prev_scratch[:],
                        idxs_ap=xidx[:],
                        num_idxs=NPCP,
                        num_idxs_reg=NPCP,
                        elem_size=ind,
                        single_packet=False,
                    )
                ROW = ROWS_L[li]
                # combined table rows [feat bf16 | el bf16 | junk]
                tabsb = bigpool.tile([P, xj * ROW], bf16, tag="tabsb")
                nc.vector.memset(tabsb[:], 0.0)
                er_own = sb.tile([P, xj * H], bf16, tag="er_own")
                for i in range(xj):
                    xT_ps = psA.tile([P, P], f32, tag="psA")
                    nc.tensor.transpose(
                        out=xT_ps[:], in_=x_own[:, i * ind:(i + 1) * ind],
                        identity=ident[:])
                    xT = sb.tile([P, ind], f32, tag="xT")
                    nc.any.tensor_copy(xT[:], xT_ps[:, :ind])
                    f_ps = psA.tile([P, hd + 2 * H], f32, tag="psA")
                    nc.tensor.matmul(out=f_ps[:, :hd], lhsT=xT[:],
                                     rhs=Ws[li][:], start=True, stop=True)
                    nc.tensor.matmul(out=f_ps[:, hd:hd + H], lhsT=xT[:],
                                     rhs=Wals[li][:], start=True, stop=True)
                    nc.tensor.matmul(out=f_ps[:, hd + H:hd + 2 * H],
                                     lhsT=xT[:], rhs=Wars[li][:],
                                     start=True, stop=True)
                    nc.any.tensor_copy(
                        tabsb[:, i * ROW:i * ROW + hd], f_ps[:, :hd])
                    nc.any.tensor_copy(
                        tabsb[:, i * ROW + hd:i * ROW + hd + H],
                        f_ps[:, hd:hd + H])
                    nc.any.tensor_copy(er_own[:, i * H:(i + 1) * H],
                                       f_ps[:, hd + H:hd + 2 * H])

                tab_own_d = dram.tile([NPCP, ROW], bf16, tag=f"tab_own{li}")
                nc.sync.dma_start(
                    tab_own_d[:].rearrange("(i p) d -> p i d", p=P),
                    tabsb[:].rearrange("p (i d) -> p i d", d=ROW))
                er_own_d = dram.tile([NPCP, 128], bf16, tag=f"er_own{li}")
                nc.sync.dma_start(
                    er_own_d[:, :H].rearrange("(i p) h -> p i h", p=P),
                    er_own[:].rearrange("p (i h) -> p i h", h=H))

                # ---------------- all-gather ----------------
                tab_full = dram.tile([n_nodes, ROW], bf16,
                                     addr_space="Shared", tag=f"tab_full{li}")
                if n_cores == 1:
                    nc.sync.dma_start(tab_full[:], tab_own_d[:npc, :])
                else:
                    nc.gpsimd.collective_compute(
                        "AllGather", mybir.AluOpType.bypass,
                        replica_groups=[list(range(n_cores))],
                        ins=[tab_own_d[:npc, :]],
                        outs=[tab_full[:]],
                    )

                # ---------------- edge phase ----------------
                if li < 2:
                    scratch = dram.tile([ng * P, hd], f32, tag=f"scratch{li}")
                else:
                    scratch = None

                for k in range(nchunk):
                    g0 = k * CHUNK_GROUPS
                    g1 = g0 + CHUNK_GROUPS
                    lt0, lt1 = int(lo_base[g0]), int(lo_base[g1])
                    ht0, ht1 = int(hi_base[g0]), int(hi_base[g1])
                    zones = []
                    if lt1 > lt0:
                        zones.append(("lo", lt0, lt1, idx_lo, ier_lo, dr_lo,
                                      tab_full[:SPLIT, :]
                                      if n_nodes > SPLIT else tab_full[:]))
                    if ht1 > ht0:
                        zones.append(("hi", ht0, ht1, idx_hi, ier_hi, dr_hi,
                                      tab_full[SPLIT:, :]))
                    ztiles = {}
                    for (zn, tt0, tt1, zidx, zier, zdr, tab_ap) in zones:
                        ntile = tt1 - tt0
                        ni = ntile * P
                        fgt = fgpool.tile([P, ntile * ROW], bf16,
                                          tag=f"fg_{zn}")
                        nc.gpsimd.dma_gather(
                            out_ap=fgt[:].rearrange("p (j d) -> p j d",
                                                    d=ROW),
                            in_ap=tab_ap,
                            idxs_ap=zidx[:, tt0 * 8:tt1 * 8],
                            num_idxs=ni,
                            num_idxs_reg=ni,
                            elem_size=ROW,
                            single_packet=False,
                        )
                        erg = fgpool.tile([P, ntile * 128], bf16,
                                          tag=f"er_{zn}")
                        nc.gpsimd.dma_gather(
                            out_ap=erg[:].rearrange("p (j d) -> p j d",
                                                    d=128),
                            in_ap=er_own_d[:],
                            idxs_ap=zier[:, tt0 * 8:tt1 * 8],
                            num_idxs=ni,
                            num_idxs_reg=ni,
                            elem_size=128,
                            single_packet=False,
                        )
                        # w = exp(leaky_relu(el + er))
                        esum = wpool.tile([P, ntile * H], f32,
                                          tag=f"esum_{zn}")
                        nc.vector.tensor_tensor(
                            out=esum[:].rearrange("p (j h) -> p j h", h=H),
                            in0=fgt[:].rearrange("p (j d) -> p j d",
                                                 d=ROW)[:, :, hd:hd + H],
                            in1=erg[:].rearrange("p (j d) -> p j d",
                                                 d=128)[:, :, :H],
                            op=OP.add)
                        esc = wpool.tile([P, ntile * H], f32, tag=f"esc_{zn}")
                        nc.vector.tensor_scalar_mul(esc[:], esum[:],
                                                    NEG_SLOPE)
                        nc.vector.tensor_tensor(out=esum[:], in0=esum[:],
                                                in1=esc[:], op=OP.max)
                        wch = wpool.tile([P, ntile * H], bf16, tag=f"w_{zn}")
                        nc.scalar.activation(wch[:], esum[:], AF.Exp)
                        ztiles[zn] = (fgt, wch, tt0)

                    for g in range(g0, g1):
                        nt_tot = (int(lo_base[g + 1]) - int(lo_base[g]) +
                                  int(hi_base[g + 1]) - int(hi_base[g]))
                        dst_ap = (scratch[g * P:(g + 1) * P, :] if scratch
                                  is not None
                                  else t_out[g * P:(g + 1) * P, :])
                        if nt_tot == 0:
                            # group covers only empty dsts: out = (relu(b))
                            ot = sb.tile([P, hd], f32, tag="ot")
                            if apply_relu:
                                nc.vector.tensor_scalar_max(
                                    ot[:], Bs[li][:, :hd], 0.0)
                            else:
                                nc.vector.tensor_copy(ot[:], Bs[li][:, :hd])
                            nc.sync.dma_start(dst_ap, ot[:])
                            continue
                        ps = pspool.tile([P, hd], f32, tag="ps")
                        psd = psdpool.tile([P, H], f32, tag="psd")
                        first = True
                        done = 0
                        for zn, zb0, zb1, zdr in (
                            ("lo", int(lo_base[g]), int(lo_base[g + 1]),
                             dr_lo),
                            ("hi", int(hi_base[g]), int(hi_base[g + 1]),
                             dr_hi),
                        ):
                            ntg = zb1 - zb0
                            if ntg == 0:
                                continue
                            fgt, wch, tt0 = ztiles[zn]
                            rel = zb0 - tt0
                            oh = sb.tile([P, ntg * P], bf16, tag="oh")
                            nc.vector.tensor_tensor(
                                out=oh[:].rearrange("p (t d) -> p t d", d=P),
                                in0=_midb(iota[:], ntg),
                                in1=zdr[:, zb0:zb1].to_broadcast([P, ntg, P]),
                                op=OP.is_equal)
                            mg = sb.tile([P, ntg * hd], bf16, tag="mg")
                            nc.vector.tensor_tensor(
                                out=mg[:].rearrange("p (t h d) -> p t h d",
                                                    h=H, d=D),
                                in0=fgt[:].rearrange("p (t d) -> p t d",
                                                     d=ROW)[:, rel:rel + ntg,
                                                            :hd]
                                    .rearrange("p t (h d) -> p t h d", d=D),
                                in1=wch[:, rel * H:(rel + ntg) * H]
                                    .rearrange("p (t h) -> p t h", h=H)
                                    .to_broadcast([P, ntg, H, D]),
                                op=OP.mult)
                            for t in range(ntg):
                                done += 1
                                nc.tensor.matmul(
                                    out=ps[:],
                                    lhsT=oh[:, t * P:(t + 1) * P],
                                    rhs=mg[:, t * hd:(t + 1) * hd],
                                    start=first, stop=(done == nt_tot))
                                nc.tensor.matmul(
                                    out=psd[:],
                                    lhsT=oh[:, t * P:(t + 1) * P],
                                    rhs=wch[:, (rel + t) * H:
                                            (rel + t + 1) * H],
                                    start=first, stop=(done == nt_tot))
                                first = False
                        # epilogue: divide, bias, relu
                        den = sb.tile([P, H], f32, tag="den")
                        nc.vector.tensor_scalar_max(den[:], psd[:], 1e-12)
                        rec = sb.tile([P, H], f32, tag="rec")
                        nc.vector.reciprocal(rec[:], den[:])
                        ot = sb.tile([P, hd], f32, tag="ot")
                        nc.vector.tensor_tensor(
                            out=ot[:].rearrange("p (h d) -> p h d", d=D),
                            in0=ps[:].rearrange("p (h d) -> p h d", d=D),
                            in1=rec[:].to_broadcast([P, H, D]),
                            op=OP.mult)
                        nc.vector.tensor_tensor(out=ot[:], in0=ot[:],
                                                in1=Bs[li][:, :hd], op=OP.add)
                        if apply_relu:
                            nc.vector.tensor_scalar_max(ot[:], ot[:], 0.0)
                        nc.sync.dma_start(dst_ap, ot[:])
                prev_scratch = scratch
    nc.compile()
    return nc


# ============================ entry point ====================================

def _meta_key(meta):
    return (meta["ng"], meta["TL"], meta["TH"], meta["SL"], meta["SH"],
            meta["xj"], meta["npc"], meta["n_nodes"], meta["n_cores"])


def _get_compiled(meta):
    key = _meta_key(meta)
    if key not in _cache:
        _cache[key] = _build_program(meta)
    return _cache[key]


def _make_in_maps(inputs, meta, per_core):
    f32 = np.float32
    xj, npc = meta["xj"], meta["npc"]
    n_cores = meta["n_cores"]
    iota_rep = np.tile(np.arange(P, dtype=f32).astype(_BF16), (P, 1))
    ident = np.eye(P, dtype=f32)
    common = {"iota_rep": iota_rep, "identity": ident}
    for li in range(len(LAYERS)):
        ind, H, D, _ = LAYERS[li]
        W = np.asarray(inputs[f"W{li}"], f32)
        al = np.asarray(inputs[f"al{li}"], f32)
        ar = np.asarray(inputs[f"ar{li}"], f32)
        b = np.asarray(inputs[f"b{li}"], f32)
        hd = H * D
        al_flat = np.zeros((hd, H), f32)
        ar_flat = np.zeros((hd, H), f32)
        for h in range(H):
            al_flat[h * D:(h + 1) * D, h] = al[h]
            ar_flat[h * D:(h + 1) * D, h] = ar[h]
        common[f"W{li}"] = W
        common[f"Wal{li}"] = (W @ al_flat).astype(f32)
        common[f"War{li}"] = (W @ ar_flat).astype(f32)
        common[f"br{li}"] = np.tile(b[None, :], (P, 1)).astype(f32)

    feats = np.asarray(inputs["features"], f32)
    in_maps = []
    for c in range(n_cores):
        pc = per_core[c]
        fo = np.zeros((xj * P, 128), f32)
        fo[:npc] = feats[c * npc:(c + 1) * npc]
        in_maps.append({
            **common,
            "features_own": fo,
            "idx_lo": pc["idx_lo"], "idx_hi": pc["idx_hi"],
            "idx_er_lo": pc["idx_er_lo"], "idx_er_hi": pc["idx_er_hi"],
            "dr_lo": pc["dr_lo"], "dr_hi": pc["dr_hi"],
            "x_idx": pc["x_idx"],
        })
    return in_maps


def kernel(**inputs):
    from concourse import bass_utils

    src = np.asarray(inputs["src"]).astype(np.int64)
    dst = np.asarray(inputs["dst"]).astype(np.int64)
    n_nodes = np.asarray(inputs["features"]).shape[0]
    meta, per_core = _preprocess(src, dst, n_nodes, N_CORES)
    nc = _get_compiled(meta)
    in_maps = _make_in_maps(inputs, meta, per_core)
    n_cores = meta["n_cores"]
    res = bass_utils.run_bass_kernel_spmd(
        nc, in_maps, core_ids=list(range(n_cores)),
        trace=bool(last_run_info.get("trace", False)))
    last_run_info["exec_time_ns"] = res.exec_time_ns
    last_run_info["profile_json"] = res.profile_json
    last_run_info["res"] = res

    npc = meta["npc"]
    out = np.empty((n_nodes, OUT_DIM), np.float32)
    for c in range(n_cores):
        rows = per_core[c]["srow"]
        out[c * npc:(c + 1) * npc] = res.results[c]["out"][rows]
    return out


# revision 11
# speedup vs baseline: 1.5164x; 1.5164x over previous
"""3-layer GAT (graph attention network) on Trainium2 — Bass/Tile, 8-core SPMD.

Sharding: nodes are partitioned into 8 contiguous ranges (graph/data
parallel).  Each core owns the edges whose *destination* falls in its range.
Per layer:
  phase A : feat = x @ W for the core's own node slice (PE), attention logits
            el = feat.al and er = feat.ar computed as x @ (W@al_flat) on PE.
            A combined per-node table row [feat(bf16) | el(bf16) | pad] is
            written so ONE gather fetches both.
  AllGather the combined table so every core can gather arbitrary src rows.
  edge    : bulk dma_gather of table[src] (el comes along free) and of
            er[dst]; w = exp(leaky_relu(el+er)); segment-sum of w*feat and w
            over destination nodes via PE matmuls with on-device-built
            one-hot matrices accumulated in PSUM; epilogue divides by the
            summed w (edge softmax), adds bias, applies relu.
Edge softmax skips the segment-max subtraction: alpha = exp(e)/sum(exp(e))
is mathematically identical and the logits here are O(1).

dma_gather uses int16 indices (max 32767), so edges are split per PSUM group
into a "lo" zone (src < 32768) and a "hi" zone (src >= 32768, gathered from a
base-offset view of the table).  All indices are valid (pads point at row 0
and are killed by a -1 dstrel -> all-zero one-hot column), so descriptor
counts are compile-time constants — one SPMD program serves all 8 cores; all
data-dependent structure lives in host-built index tables.
"""

import numpy as np

try:
    import ml_dtypes
    _BF16 = ml_dtypes.bfloat16
except ImportError:  # pragma: no cover
    _BF16 = None

# ---------------- static problem config (self-contained) ---------------------
N_CORES = 8
NEG_SLOPE = 0.2
P = 128
GROUP_E = 1024             # max edges per PSUM accumulation group
CHUNK_GROUPS = 4           # groups per gather chunk
SPLIT = 32768              # int16 index split point
# (in_dim, H, D, apply_relu) per layer
LAYERS = [(128, 4, 32, True), (128, 4, 32, True), (128, 1, 64, False)]
OUT_DIM = 64
ROWS_L = (256, 256, 128)   # combined table row per layer, bf16 elems
EL_OFF = None              # el lives right after feat (offset = hd)

_cache = {}
last_run_info = {}


# ============================ host-side preprocessing ========================

def _wrap16(vals, cols):
    """dma_gather index layout: entry i -> [i % 16, i // 16], replicated
    across the 8 groups of 16 partitions."""
    t = np.zeros((16, cols), np.int16)
    n = len(vals)
    t[np.arange(n) % 16, np.arange(n) // 16] = vals.astype(np.int16)
    return np.tile(t, (8, 1))


def _preprocess(src, dst, n_nodes, n_cores):
    npc = n_nodes // n_cores
    xj = (npc + P - 1) // P
    cores = []
    for c in range(n_cores):
        lo = c * npc
        m = (dst >= lo) & (dst < lo + npc)
        s = src[m].astype(np.int64)
        d = (dst[m] - lo).astype(np.int64)
        o = np.argsort(d, kind="stable")
        s, d = s[o], d[o]
        counts = np.bincount(d, minlength=npc)
        cum = np.zeros(npc + 1, np.int64)
        np.cumsum(counts, out=cum[1:])
        groups = []
        base = 0
        while base < npc:
            dmax = min(base + P, npc)
            limit = cum[base] + GROUP_E
            dend = int(np.searchsorted(cum, limit, side="right")) - 1
            dend = min(dend, dmax)
            if dend <= base:
                raise ValueError(f"dst {base} has degree > {GROUP_E}")
            groups.append((base, int(cum[base]), int(cum[dend])))
            base = dend
        cores.append((s, d, groups))

    ng = max(len(g) for (_, _, g) in cores)
    ng = ((ng + CHUNK_GROUPS - 1) // CHUNK_GROUPS) * CHUNK_GROUPS
    # per-group-index lo/hi tile counts (max across cores -> shared program)
    TL = np.zeros(ng, np.int64)
    TH = np.zeros(ng, np.int64)
    for (s, d, groups) in cores:
        for gi, (b, e0, e1) in enumerate(groups):
            nlo = int((s[e0:e1] < SPLIT).sum())
            nhi = (e1 - e0) - nlo
            TL[gi] = max(TL[gi], (nlo + P - 1) // P)
            TH[gi] = max(TH[gi], (nhi + P - 1) // P)
    lo_base = np.zeros(ng + 1, np.int64)   # tile offsets within lo zone
    hi_base = np.zeros(ng + 1, np.int64)
    np.cumsum(TL, out=lo_base[1:])
    np.cumsum(TH, out=hi_base[1:])
    SL = int(lo_base[ng]) * P              # lo-zone slots
    SH = int(hi_base[ng]) * P
    SLP = max(SL, 2048)                    # padded sizes for tensor shapes
    SHP = max(SH, 2048)

    per_core = []
    for c, (s, d, groups) in enumerate(cores):
        idx_lo = np.zeros(SL, np.int64)
        idx_hi = np.zeros(SH, np.int64)
        er_lo = np.zeros(SL, np.int64)
        er_hi = np.zeros(SH, np.int64)
        dr_lo = np.full(SL, -1.0, np.float32)
        dr_hi = np.full(SH, -1.0, np.float32)
        for gi, (b, e0, e1) in enumerate(groups):
            eg_s = s[e0:e1]
            eg_d = d[e0:e1]
            lm = eg_s < SPLIT
            for zone, msk in ((0, lm), (1, ~lm)):
                zs = eg_s[msk]
                zd = eg_d[msk]
                n = len(zs)
                o = (int(lo_base[gi]) if zone == 0 else int(hi_base[gi])) * P
                tgt_i = idx_lo if zone == 0 else idx_hi
                tgt_e = er_lo if zone == 0 else er_hi
                tgt_r = dr_lo if zone == 0 else dr_hi
                tgt_i[o:o + n] = zs - (0 if zone == 0 else SPLIT)
                tgt_e[o:o + n] = zd
                tgt_r[o:o + n] = (zd - b).astype(np.float32)
        # scratch-row map
        srow = np.zeros(npc, np.int64)
        for gi, (b, e0, e1) in enumerate(groups):
            b_next = groups[gi + 1][0] if gi + 1 < len(groups) else npc
            srow[b:b_next] = gi * P + (np.arange(b, b_next) - b)
        nn = np.arange(xj * P)
        xi = np.zeros(xj * P, np.int64)
        valid = nn < npc
        xi[valid] = srow[nn[valid]]
        def _padcols(a, cols):
            out = np.zeros((a.shape[0], cols), a.dtype)
            out[:, :a.shape[1]] = a
            return out

        dl = dr_lo.reshape(SL // P, P).T.astype(_BF16) if SL else \
            np.zeros((P, 0), _BF16)
        dh = dr_hi.reshape(SH // P, P).T.astype(_BF16) if SH else \
            np.zeros((P, 0), _BF16)
        per_core.append(dict(
            idx_lo=_padcols(_wrap16(idx_lo, max(SL // 16, 1)), SLP // 16),
            idx_hi=_padcols(_wrap16(idx_hi, max(SH // 16, 1)), SHP // 16),
            idx_er_lo=_padcols(_wrap16(er_lo, max(SL // 16, 1)), SLP // 16),
            idx_er_hi=_padcols(_wrap16(er_hi, max(SH // 16, 1)), SHP // 16),
            dr_lo=_padcols(dl, SLP // P),
            dr_hi=_padcols(dh, SHP // P),
            x_idx=_wrap16(xi, (xj * P) // 16),
            srow=srow,
        ))
    meta = dict(ng=ng, TL=tuple(int(x) for x in TL),
                TH=tuple(int(x) for x in TH), SL=SL, SH=SH,
                SLP=SLP, SHP=SHP,
                xj=xj, npc=npc, n_nodes=n_nodes, n_cores=n_cores)
    return meta, per_core


# ============================ device program =================================

def _build_program(meta):
    import concourse.bass as bass
    import concourse.tile as tile
    from concourse import bacc, mybir

    def _midb(ap, n):
        # [P, D] -> [P, n, D] with the middle dim broadcast (step 0)
        return bass.AP(ap.tensor, ap.offset,
                       [list(ap.ap[0]), [0, n], list(ap.ap[1])])

    f32 = mybir.dt.float32
    bf16 = mybir.dt.bfloat16
    i16 = mybir.dt.int16
    AF = mybir.ActivationFunctionType
    OP = mybir.AluOpType

    ng, SL, SH = meta["ng"], meta["SL"], meta["SH"]
    SLP, SHP = meta["SLP"], meta["SHP"]
    TL, TH = meta["TL"], meta["TH"]
    xj, npc = meta["xj"], meta["npc"]
    n_nodes, n_cores = meta["n_nodes"], meta["n_cores"]
    NPCP = xj * P
    lo_base = np.concatenate([[0], np.cumsum(TL)]).astype(int)
    hi_base = np.concatenate([[0], np.cumsum(TH)]).astype(int)
    nchunk = ng // CHUNK_GROUPS

    nc = bacc.Bacc("TRN2", target_bir_lowering=False, debug=False,
                   enable_asserts=False, num_devices=n_cores,
                   num_swdge_queues=4)

    _qctr = [0]

    def _gather(out_ap3, in_ap, idxs2, ni, elem, piece=3072):
        """dma_gather split into <=piece-idx sub-gathers cycling over the 4
        SWDGE queues so descriptor generation overlaps DMA drain."""
        ntile = ni // P
        pt = max(piece // P, 1)
        for j0 in range(0, ntile, pt):
            j1 = min(j0 + pt, ntile)
            n = (j1 - j0) * P
            nc.gpsimd.dma_gather(
                out_ap=out_ap3[:, j0:j1, :],
                in_ap=in_ap,
                idxs_ap=idxs2[:, j0 * 8:j1 * 8],
                num_idxs=n,
                num_idxs_reg=n,
                elem_size=elem,
                single_packet=False,
                queue_num=_qctr[0] % 4,
            )
            _qctr[0] += 1

    t_feats = nc.dram_tensor("features_own", [NPCP, 128], f32,
                             kind="ExternalInput").ap()
    t_idx_lo = nc.dram_tensor("idx_lo", [P, SLP // 16], i16,
                              kind="ExternalInput").ap()
    t_idx_hi = nc.dram_tensor("idx_hi", [P, SHP // 16], i16,
                              kind="ExternalInput").ap()
    t_ier_lo = nc.dram_tensor("idx_er_lo", [P, SLP // 16], i16,
                              kind="ExternalInput").ap()
    t_ier_hi = nc.dram_tensor("idx_er_hi", [P, SHP // 16], i16,
                              kind="ExternalInput").ap()
    t_dr_lo = nc.dram_tensor("dr_lo", [P, SLP // P], bf16,
                             kind="ExternalInput").ap()
    t_dr_hi = nc.dram_tensor("dr_hi", [P, SHP // P], bf16,
                             kind="ExternalInput").ap()
    t_x_idx = nc.dram_tensor("x_idx", [P, NPCP // 16], i16,
                             kind="ExternalInput").ap()
    t_iota = nc.dram_tensor("iota_rep", [P, P], bf16,
                            kind="ExternalInput").ap()
    t_ident = nc.dram_tensor("identity", [P, P], f32,
                             kind="ExternalInput").ap()
    t_W, t_Wal, t_War, t_b = [], [], [], []
    for li, (ind, H, D, _) in enumerate(LAYERS):
        hd = H * D
        t_W.append(nc.dram_tensor(f"W{li}", [ind, hd], f32,
                                  kind="ExternalInput").ap())
        t_Wal.append(nc.dram_tensor(f"Wal{li}", [ind, H], f32,
                                    kind="ExternalInput").ap())
        t_War.append(nc.dram_tensor(f"War{li}", [ind, H], f32,
                                    kind="ExternalInput").ap())
        t_b.append(nc.dram_tensor(f"br{li}", [P, hd], f32,
                                  kind="ExternalInput").ap())
    t_out = nc.dram_tensor("out", [ng * P, OUT_DIM], f32,
                           kind="ExternalOutput").ap()

    with tile.TileContext(nc) as tc:
        with (
            tc.tile_pool(name="const", bufs=1) as cpool,
            tc.tile_pool(name="big", bufs=1) as bigpool,
            tc.tile_pool(name="sb", bufs=3) as sb,
            tc.tile_pool(name="fg", bufs=2) as fgpool,
            tc.tile_pool(name="wp", bufs=2) as wpool,
            tc.tile_pool(name="ps", bufs=3, space="PSUM") as pspool,
            tc.tile_pool(name="psd", bufs=3, space="PSUM") as psdpool,
            tc.tile_pool(name="psA", bufs=2, space="PSUM") as psA,
            tc.tile_pool(name="dram", bufs=1, space="DRAM") as dram,
        ):
            # ---- constants ----
            ident = cpool.tile([P, P], f32)
            nc.sync.dma_start(ident[:], t_ident)
            iota = cpool.tile([P, P], bf16)
            nc.sync.dma_start(iota[:], t_iota)
            idx_lo = cpool.tile([P, SLP // 16], i16)
            nc.sync.dma_start(idx_lo[:], t_idx_lo)
            idx_hi = cpool.tile([P, SHP // 16], i16)
            nc.sync.dma_start(idx_hi[:], t_idx_hi)
            ier_lo = cpool.tile([P, SLP // 16], i16)
            nc.sync.dma_start(ier_lo[:], t_ier_lo)
            ier_hi = cpool.tile([P, SHP // 16], i16)
            nc.sync.dma_start(ier_hi[:], t_ier_hi)
            dr_lo = cpool.tile([P, SLP // P], bf16)
            nc.sync.dma_start(dr_lo[:], t_dr_lo)
            dr_hi = cpool.tile([P, SHP // P], bf16)
            nc.sync.dma_start(dr_hi[:], t_dr_hi)
            xidx = cpool.tile([P, NPCP // 16], i16)
            nc.sync.dma_start(xidx[:], t_x_idx)
            Ws, Wals, Wars, Bs = [], [], [], []
            for li, (ind, H, D, _) in enumerate(LAYERS):
                hd = H * D
                w = cpool.tile([ind, hd], f32, tag=f"W{li}")
                nc.sync.dma_start(w[:], t_W[li])
                Ws.append(w)
                wa = cpool.tile([ind, H], f32, tag=f"Wal{li}")
                nc.sync.dma_start(wa[:], t_Wal[li])
                Wals.append(wa)
                wr = cpool.tile([ind, H], f32, tag=f"War{li}")
                nc.sync.dma_start(wr[:], t_War[li])
                Wars.append(wr)
                bb = cpool.tile([P, hd], f32, tag=f"br{li}")
                nc.sync.dma_start(bb[:], t_b[li])
                Bs.append(bb)

            prev_scratch = None
            for li, (ind, H, D, apply_relu) in enumerate(LAYERS):
                hd = H * D
                # ---------------- phase A ----------------
                x_own = bigpool.tile([P, xj * ind], f32, tag="x_own")
                if li == 0:
                    nc.sync.dma_start(
                        x_own[:].rearrange("p (i d) -> p i d", d=ind),
                        t_feats.rearrange("(i p) d -> p i d", p=P))
                else:
                    _gather(x_own[:].rearrange("p (i d) -> p i d", d=ind),
                            prev_scratch[:], xidx[:], NPCP, ind)
                ROW = ROWS_L[li]
                # combined table rows [feat bf16 | el bf16 | junk]
                tabsb = bigpool.tile([P, xj * ROW], bf16, tag="tabsb")
                nc.vector.memset(tabsb[:], 0.0)
                er_own = sb.tile([P, xj * H], bf16, tag="er_own")
                for i in range(xj):
                    xT_ps = psA.tile([P, P], f32, tag="psA")
                    nc.tensor.transpose(
                        out=xT_ps[:], in_=x_own[:, i * ind:(i + 1) * ind],
                        identity=ident[:])
                    xT = sb.tile([P, ind], f32, tag="xT")
                    nc.any.tensor_copy(xT[:], xT_ps[:, :ind])
                    f_ps = psA.tile([P, hd + 2 * H], f32, tag="psA")
                    nc.tensor.matmul(out=f_ps[:, :hd], lhsT=xT[:],
                                     rhs=Ws[li][:], start=True, stop=True)
                    nc.tensor.matmul(out=f_ps[:, hd:hd + H], lhsT=xT[:],
                                     rhs=Wals[li][:], start=True, stop=True)
                    nc.tensor.matmul(out=f_ps[:, hd + H:hd + 2 * H],
                                     lhsT=xT[:], rhs=Wars[li][:],
                                     start=True, stop=True)
                    nc.any.tensor_copy(
                        tabsb[:, i * ROW:i * ROW + hd], f_ps[:, :hd])
                    nc.any.tensor_copy(
                        tabsb[:, i * ROW + hd:i * ROW + hd + H],
                        f_ps[:, hd:hd + H])
                    nc.any.tensor_copy(er_own[:, i * H:(i + 1) * H],
                                       f_ps[:, hd + H:hd + 2 * H])

                tab_own_d = dram.tile([NPCP, ROW], bf16, tag=f"tab_own{li}")
                nc.sync.dma_start(
                    tab_own_d[:].rearrange("(i p) d -> p i d", p=P),
                    tabsb[:].rearrange("p (i d) -> p i d", d=ROW))
                er_own_d = dram.tile([NPCP, 128], bf16, tag=f"er_own{li}")
                nc.sync.dma_start(
                    er_own_d[:, :H].rearrange("(i p) h -> p i h", p=P),
                    er_own[:].rearrange("p (i h) -> p i h", h=H))

                # ---------------- all-gather ----------------
                tab_full = dram.tile([n_nodes, ROW], bf16,
                                     addr_space="Shared", tag=f"tab_full{li}")
                if n_cores == 1:
                    nc.sync.dma_start(tab_full[:], tab_own_d[:npc, :])
                else:
                    nc.gpsimd.collective_compute(
                        "AllGather", mybir.AluOpType.bypass,
                        replica_groups=[list(range(n_cores))],
                        ins=[tab_own_d[:npc, :]],
                        outs=[tab_full[:]],
                    )

                # ---------------- edge phase ----------------
                if li < 2:
                    scratch = dram.tile([ng * P, hd], f32, tag=f"scratch{li}")
                else:
                    scratch = None

                for k in range(nchunk):
                    g0 = k * CHUNK_GROUPS
                    g1 = g0 + CHUNK_GROUPS
                    lt0, lt1 = int(lo_base[g0]), int(lo_base[g1])
                    ht0, ht1 = int(hi_base[g0]), int(hi_base[g1])
                    zones = []
                    if lt1 > lt0:
                        zones.append(("lo", lt0, lt1, idx_lo, ier_lo, dr_lo,
                                      tab_full[:SPLIT, :]
                                      if n_nodes > SPLIT else tab_full[:]))
                    if ht1 > ht0:
                        zones.append(("hi", ht0, ht1, idx_hi, ier_hi, dr_hi,
                                      tab_full[SPLIT:, :]))
                    ztiles = {}
                    for (zn, tt0, tt1, zidx, zier, zdr, tab_ap) in zones:
                        ntile = tt1 - tt0
                        ni = ntile * P
                        fgt = fgpool.tile([P, ntile * ROW], bf16,
                                          tag=f"fg_{zn}")
                        _gather(fgt[:].rearrange("p (j d) -> p j d", d=ROW),
                                tab_ap, zidx[:, tt0 * 8:tt1 * 8], ni, ROW)
                        erg = fgpool.tile([P, ntile * 128], bf16,
                                          tag=f"er_{zn}")
                        _gather(erg[:].rearrange("p (j d) -> p j d", d=128),
                                er_own_d[:], zier[:, tt0 * 8:tt1 * 8],
                                ni, 128)
                        # w = exp(leaky_relu(el + er))
                        esum = wpool.tile([P, ntile * H], f32,
                                          tag=f"esum_{zn}")
                        nc.vector.tensor_tensor(
                            out=esum[:].rearrange("p (j h) -> p j h", h=H),
                            in0=fgt[:].rearrange("p (j d) -> p j d",
                                                 d=ROW)[:, :, hd:hd + H],
                            in1=erg[:].rearrange("p (j d) -> p j d",
                                                 d=128)[:, :, :H],
                            op=OP.add)
                        esc = wpool.tile([P, ntile * H], f32, tag=f"esc_{zn}")
                        nc.vector.tensor_scalar_mul(esc[:], esum[:],
                                                    NEG_SLOPE)
                        nc.vector.tensor_tensor(out=esum[:], in0=esum[:],
                                                in1=esc[:], op=OP.max)
                        wch = wpool.tile([P, ntile * H], bf16, tag=f"w_{zn}")
                        nc.scalar.activation(wch[:], esum[:], AF.Exp)
                        ztiles[zn] = (fgt, wch, tt0)

                    for g in range(g0, g1):
                        nt_tot = (int(lo_base[g + 1]) - int(lo_base[g]) +
                                  int(hi_base[g + 1]) - int(hi_base[g]))
                        dst_ap = (scratch[g * P:(g + 1) * P, :] if scratch
                                  is not None
                                  else t_out[g * P:(g + 1) * P, :])
                        if nt_tot == 0:
                            # group covers only empty dsts: out = (relu(b))
                            ot = sb.tile([P, hd], f32, tag="ot")
                            if apply_relu:
                                nc.vector.tensor_scalar_max(
                                    ot[:], Bs[li][:, :hd], 0.0)
                            else:
                                nc.vector.tensor_copy(ot[:], Bs[li][:, :hd])
                            nc.sync.dma_start(dst_ap, ot[:])
                            continue
                        ps = pspool.tile([P, hd], f32, tag="ps")
                        psd = psdpool.tile([P, H], f32, tag="psd")
                        first = True
                        done = 0
                        for zn, zb0, zb1, zdr in (
                            ("lo", int(lo_base[g]), int(lo_base[g + 1]),
                             dr_lo),
                            ("hi", int(hi_base[g]), int(hi_base[g + 1]),
                             dr_hi),
                        ):
                            ntg = zb1 - zb0
                            if ntg == 0:
                                continue
                            fgt, wch, tt0 = ztiles[zn]
                            rel = zb0 - tt0
                            oh = sb.tile([P, ntg * P], bf16, tag="oh")
                            nc.vector.tensor_tensor(
                                out=oh[:].rearrange("p (t d) -> p t d", d=P),
                                in0=_midb(iota[:], ntg),
                                in1=zdr[:, zb0:zb1].to_broadcast([P, ntg, P]),
                                op=OP.is_equal)
                            mg = sb.tile([P, ntg * hd], bf16, tag="mg")
                            nc.vector.tensor_tensor(
                                out=mg[:].rearrange("p (t h d) -> p t h d",
                                                    h=H, d=D),
                                in0=fgt[:].rearrange("p (t d) -> p t d",
                                                     d=ROW)[:, rel:rel + ntg,
                                                            :hd]
                                    .rearrange("p t (h d) -> p t h d", d=D),
                                in1=wch[:, rel * H:(rel + ntg) * H]
                                    .rearrange("p (t h) -> p t h", h=H)
                                    .to_broadcast([P, ntg, H, D]),
                                op=OP.mult)
                            for t in range(ntg):
                                done += 1
                                nc.tensor.matmul(
                                    out=ps[:],
                                    lhsT=oh[:, t * P:(t + 1) * P],
                                    rhs=mg[:, t * hd:(t + 1) * hd],
                                    start=first, stop=(done == nt_tot))
                                nc.tensor.matmul(
                                    out=psd[:],
                                    lhsT=oh[:, t * P:(t + 1) * P],
                                    rhs=wch[:, (rel + t) * H:
                                            (rel + t + 1) * H],
                                    start=first, stop=(done == nt_tot))
                                first = False
                        # epilogue: divide, bias, relu
                        den = sb.tile([P, H], f32, tag="den")
                        nc.vector.tensor_scalar_max(den[:], psd[:], 1e-12)
                        rec = sb.tile([P, H], f32, tag="rec")
                        nc.vector.reciprocal(rec[:], den[:])
                        ot = sb.tile([P, hd], f32, tag="ot")
                        nc.vector.tensor_tensor(
                            out=ot[:].rearrange("p (h d) -> p h d", d=D),
                            in0=ps[:].rearrange("p (h d) -> p h d", d=D),
                            in1=rec[:].to_broadcast([P, H, D]),
                            op=OP.mult)
                        nc.vector.tensor_tensor(out=ot[:], in0=ot[:],
                                                in1=Bs[li][:, :hd], op=OP.add)
                        if apply_relu:
                            nc.vector.tensor_scalar_max(ot[:], ot[:], 0.0)
                        nc.sync.dma_start(dst_ap, ot[:])
                prev_scratch = scratch
    nc.compile()
    return nc


# ============================ entry point ====================================

def _meta_key(meta):
    return (meta["ng"], meta["TL"], meta["TH"], meta["SL"], meta["SH"],
            meta["xj"], meta["npc"], meta["n_nodes"], meta["n_cores"])


def _get_compiled(meta):
    key = _meta_key(meta)
    if key not in _cache:
        _cache[key] = _build_program(meta)
    return _cache[key]


def _make_in_maps(inputs, meta, per_core):
    f32 = np.float32
    xj, npc = meta["xj"], meta["npc"]
    n_cores = meta["n_cores"]
    iota_rep = np.tile(np.arange(P, dtype=f32).astype(_BF16), (P, 1))
    ident = np.eye(P, dtype=f32)
    common = {"iota_rep": iota_rep, "identity": ident}
    for li in range(len(LAYERS)):
        ind, H, D, _ = LAYERS[li]
        W = np.asarray(inputs[f"W{li}"], f32)
        al = np.asarray(inputs[f"al{li}"], f32)
        ar = np.asarray(inputs[f"ar{li}"], f32)
        b = np.asarray(inputs[f"b{li}"], f32)
        hd = H * D
        al_flat = np.zeros((hd, H), f32)
        ar_flat = np.zeros((hd, H), f32)
        for h in range(H):
            al_flat[h * D:(h + 1) * D, h] = al[h]
            ar_flat[h * D:(h + 1) * D, h] = ar[h]
        common[f"W{li}"] = W
        common[f"Wal{li}"] = (W @ al_flat).astype(f32)
        common[f"War{li}"] = (W @ ar_flat).astype(f32)
        common[f"br{li}"] = np.tile(b[None, :], (P, 1)).astype(f32)

    feats = np.asarray(inputs["features"], f32)
    in_maps = []
    for c in range(n_cores):
        pc = per_core[c]
        fo = np.zeros((xj * P, 128), f32)
        fo[:npc] = feats[c * npc:(c + 1) * npc]
        in_maps.append({
            **common,
            "features_own": fo,
            "idx_lo": pc["idx_lo"], "idx_hi": pc["idx_hi"],
            "idx_er_lo": pc["idx_er_lo"], "idx_er_hi": pc["idx_er_hi"],
            "dr_lo": pc["dr_lo"], "dr_hi": pc["dr_hi"],
            "x_idx": pc["x_idx"],
        })
    return in_maps


def kernel(**inputs):
    from concourse import bass_utils

    src = np.asarray(inputs["src"]).astype(np.int64)
    dst = np.asarray(inputs["dst"]).astype(np.int64)
    n_nodes = np.asarray(inputs["features"]).shape[0]
    meta, per_core = _preprocess(src, dst, n_nodes, N_CORES)
    nc = _get_compiled(meta)
    in_maps = _make_in_maps(inputs, meta, per_core)
    n_cores = meta["n_cores"]
    res = bass_utils.run_bass_kernel_spmd(
        nc, in_maps, core_ids=list(range(n_cores)),
        trace=bool(last_run_info.get("trace", False)))
    last_run_info["exec_time_ns"] = res.exec_time_ns
    last_run_info["profile_json"] = res.profile_json
    last_run_info["res"] = res

    npc = meta["npc"]
    out = np.empty((n_nodes, OUT_DIM), np.float32)
    for c in range(n_cores):
        rows = per_core[c]["srow"]
        out[c * npc:(c + 1) * npc] = res.results[c]["out"][rows]
    return out


# revision 13
# speedup vs baseline: 1.8323x; 1.2084x over previous
"""3-layer GAT (graph attention network) on Trainium2 — Bass/Tile, 8-core SPMD.

Sharding: nodes are partitioned into 8 contiguous ranges (graph/data
parallel).  Each core owns the edges whose *destination* falls in its range.
Per layer:
  phase A : feat = x @ W for the core's own node slice (PE), attention logits
            el = feat.al and er = feat.ar computed as x @ (W@al_flat) on PE.
            A combined per-node table row [feat(bf16) | el(bf16) | pad] is
            written so ONE gather fetches both.
  AllGather the combined table so every core can gather arbitrary src rows.
  edge    : bulk dma_gather of table[src] (el comes along free) and of
            er[dst]; w = exp(leaky_relu(el+er)); segment-sum of w*feat and w
            over destination nodes via PE matmuls with on-device-built
            one-hot matrices accumulated in PSUM; epilogue divides by the
            summed w (edge softmax), adds bias, applies relu.
Edge softmax skips the segment-max subtraction: alpha = exp(e)/sum(exp(e))
is mathematically identical and the logits here are O(1).

dma_gather uses int16 indices (max 32767), so edges are split per PSUM group
into a "lo" zone (src < 32768) and a "hi" zone (src >= 32768, gathered from a
base-offset view of the table).  All indices are valid (pads point at row 0
and are killed by a -1 dstrel -> all-zero one-hot column), so descriptor
counts are compile-time constants — one SPMD program serves all 8 cores; all
data-dependent structure lives in host-built index tables.
"""

import numpy as np

try:
    import ml_dtypes
    _BF16 = ml_dtypes.bfloat16
except ImportError:  # pragma: no cover
    _BF16 = None

# ---------------- static problem config (self-contained) ---------------------
N_CORES = 8
NEG_SLOPE = 0.2
P = 128
GROUP_E = 1024             # max edges per PSUM accumulation group
CHUNK_GROUPS = 4           # groups per gather chunk
SPLIT = 32768              # int16 index split point
# (in_dim, H, D, apply_relu) per layer
LAYERS = [(128, 4, 32, True), (128, 4, 32, True), (128, 1, 64, False)]
OUT_DIM = 64
ROWS_L = (256, 256, 128)   # combined table row per layer, bf16 elems
EL_OFF = None              # el lives right after feat (offset = hd)

_cache = {}
last_run_info = {}


# ============================ host-side preprocessing ========================

def _wrap16(vals, cols):
    """dma_gather index layout: entry i -> [i % 16, i // 16], replicated
    across the 8 groups of 16 partitions."""
    t = np.zeros((16, cols), np.int16)
    n = len(vals)
    t[np.arange(n) % 16, np.arange(n) // 16] = vals.astype(np.int16)
    return np.tile(t, (8, 1))


def _preprocess(src, dst, n_nodes, n_cores):
    npc = n_nodes // n_cores
    xj = (npc + P - 1) // P
    cores = []
    for c in range(n_cores):
        lo = c * npc
        m = (dst >= lo) & (dst < lo + npc)
        s = src[m].astype(np.int64)
        d = (dst[m] - lo).astype(np.int64)
        o = np.argsort(d, kind="stable")
        s, d = s[o], d[o]
        counts = np.bincount(d, minlength=npc)
        cum = np.zeros(npc + 1, np.int64)
        np.cumsum(counts, out=cum[1:])
        groups = []
        base = 0
        while base < npc:
            dmax = min(base + P, npc)
            limit = cum[base] + GROUP_E
            dend = int(np.searchsorted(cum, limit, side="right")) - 1
            dend = min(dend, dmax)
            if dend <= base:
                raise ValueError(f"dst {base} has degree > {GROUP_E}")
            groups.append((base, int(cum[base]), int(cum[dend])))
            base = dend
        cores.append((s, d, groups))

    ng = max(len(g) for (_, _, g) in cores)
    ng = ((ng + CHUNK_GROUPS - 1) // CHUNK_GROUPS) * CHUNK_GROUPS
    # per-group-index lo/hi tile counts (max across cores -> shared program)
    TL = np.zeros(ng, np.int64)
    TH = np.zeros(ng, np.int64)
    for (s, d, groups) in cores:
        for gi, (b, e0, e1) in enumerate(groups):
            nlo = int((s[e0:e1] < SPLIT).sum())
            nhi = (e1 - e0) - nlo
            TL[gi] = max(TL[gi], (nlo + P - 1) // P)
            TH[gi] = max(TH[gi], (nhi + P - 1) // P)
    lo_base = np.zeros(ng + 1, np.int64)   # tile offsets within lo zone
    hi_base = np.zeros(ng + 1, np.int64)
    np.cumsum(TL, out=lo_base[1:])
    np.cumsum(TH, out=hi_base[1:])
    SL = int(lo_base[ng]) * P              # lo-zone slots
    SH = int(hi_base[ng]) * P
    SLP = max(SL, 2048)                    # padded sizes for tensor shapes
    SHP = max(SH, 2048)

    per_core = []
    for c, (s, d, groups) in enumerate(cores):
        idx_lo = np.zeros(SL, np.int64)
        idx_hi = np.zeros(SH, np.int64)
        er_lo = np.zeros(SL, np.int64)
        er_hi = np.zeros(SH, np.int64)
        dr_lo = np.full(SL, -1.0, np.float32)
        dr_hi = np.full(SH, -1.0, np.float32)
        for gi, (b, e0, e1) in enumerate(groups):
            eg_s = s[e0:e1]
            eg_d = d[e0:e1]
            lm = eg_s < SPLIT
            for zone, msk in ((0, lm), (1, ~lm)):
                zs = eg_s[msk]
                zd = eg_d[msk]
                n = len(zs)
                o = (int(lo_base[gi]) if zone == 0 else int(hi_base[gi])) * P
                tgt_i = idx_lo if zone == 0 else idx_hi
                tgt_e = er_lo if zone == 0 else er_hi
                tgt_r = dr_lo if zone == 0 else dr_hi
                tgt_i[o:o + n] = zs - (0 if zone == 0 else SPLIT)
                tgt_e[o:o + n] = zd
                tgt_r[o:o + n] = (zd - b).astype(np.float32)
        # scratch-row map
        srow = np.zeros(npc, np.int64)
        for gi, (b, e0, e1) in enumerate(groups):
            b_next = groups[gi + 1][0] if gi + 1 < len(groups) else npc
            srow[b:b_next] = gi * P + (np.arange(b, b_next) - b)
        nn = np.arange(xj * P)
        xi = np.zeros(xj * P, np.int64)
        valid = nn < npc
        xi[valid] = srow[nn[valid]]
        def _padcols(a, cols):
            out = np.zeros((a.shape[0], cols), a.dtype)
            out[:, :a.shape[1]] = a
            return out

        dl = dr_lo.reshape(SL // P, P).T.astype(_BF16) if SL else \
            np.zeros((P, 0), _BF16)
        dh = dr_hi.reshape(SH // P, P).T.astype(_BF16) if SH else \
            np.zeros((P, 0), _BF16)
        per_core.append(dict(
            idx_lo=_padcols(_wrap16(idx_lo, max(SL // 16, 1)), SLP // 16),
            idx_hi=_padcols(_wrap16(idx_hi, max(SH // 16, 1)), SHP // 16),
            idx_er_lo=_padcols(_wrap16(er_lo, max(SL // 16, 1)), SLP // 16),
            idx_er_hi=_padcols(_wrap16(er_hi, max(SH // 16, 1)), SHP // 16),
            dr_lo=_padcols(dl, SLP // P),
            dr_hi=_padcols(dh, SHP // P),
            x_idx=_wrap16(xi, (xj * P) // 16),
            srow=srow,
        ))
    meta = dict(ng=ng, TL=tuple(int(x) for x in TL),
                TH=tuple(int(x) for x in TH), SL=SL, SH=SH,
                SLP=SLP, SHP=SHP,
                xj=xj, npc=npc, n_nodes=n_nodes, n_cores=n_cores)
    return meta, per_core


# ============================ device program =================================

def _build_program(meta):
    import concourse.bass as bass
    import concourse.tile as tile
    from concourse import bacc, mybir

    def _midb(ap, n):
        # [P, D] -> [P, n, D] with the middle dim broadcast (step 0)
        return bass.AP(ap.tensor, ap.offset,
                       [list(ap.ap[0]), [0, n], list(ap.ap[1])])

    f32 = mybir.dt.float32
    bf16 = mybir.dt.bfloat16
    i16 = mybir.dt.int16
    AF = mybir.ActivationFunctionType
    OP = mybir.AluOpType

    ng, SL, SH = meta["ng"], meta["SL"], meta["SH"]
    SLP, SHP = meta["SLP"], meta["SHP"]
    TL, TH = meta["TL"], meta["TH"]
    xj, npc = meta["xj"], meta["npc"]
    n_nodes, n_cores = meta["n_nodes"], meta["n_cores"]
    NPCP = xj * P
    lo_base = np.concatenate([[0], np.cumsum(TL)]).astype(int)
    hi_base = np.concatenate([[0], np.cumsum(TH)]).astype(int)
    nchunk = ng // CHUNK_GROUPS

    nc = bacc.Bacc("TRN2", target_bir_lowering=False, debug=False,
                   enable_asserts=False, num_devices=n_cores,
                   num_swdge_queues=4)

    _qctr = [0]
    _qsems = [nc.alloc_semaphore(f"gdma{q}") for q in range(4)]

    def _gather(out_ap3, in_ap, idxs2, ni, elem, piece=1024):
        """dma_gather as prepare_only + trigger, split into <=piece-idx
        sub-gathers cycling over the 4 SWDGE queues, so Q7 descriptor
        generation never stalls on its own DMA drain."""
        ntile = ni // P
        pt = max(piece // P, 1)
        for j0 in range(0, ntile, pt):
            j1 = min(j0 + pt, ntile)
            n = (j1 - j0) * P
            q = _qctr[0] % 4
            nc.gpsimd.dma_gather(
                out_ap=out_ap3[:, j0:j1, :],
                in_ap=in_ap,
                idxs_ap=idxs2[:, j0 * 8:j1 * 8],
                num_idxs=n,
                num_idxs_reg=n,
                elem_size=elem,
                single_packet=False,
                queue_num=q,
            )
            _qctr[0] += 1

    t_feats = nc.dram_tensor("features_own", [NPCP, 128], f32,
                             kind="ExternalInput").ap()
    t_idx_lo = nc.dram_tensor("idx_lo", [P, SLP // 16], i16,
                              kind="ExternalInput").ap()
    t_idx_hi = nc.dram_tensor("idx_hi", [P, SHP // 16], i16,
                              kind="ExternalInput").ap()
    t_ier_lo = nc.dram_tensor("idx_er_lo", [P, SLP // 16], i16,
                              kind="ExternalInput").ap()
    t_ier_hi = nc.dram_tensor("idx_er_hi", [P, SHP // 16], i16,
                              kind="ExternalInput").ap()
    t_dr_lo = nc.dram_tensor("dr_lo", [P, SLP // P], bf16,
                             kind="ExternalInput").ap()
    t_dr_hi = nc.dram_tensor("dr_hi", [P, SHP // P], bf16,
                             kind="ExternalInput").ap()
    t_x_idx = nc.dram_tensor("x_idx", [P, NPCP // 16], i16,
                             kind="ExternalInput").ap()
    t_iota = nc.dram_tensor("iota_rep", [P, P], bf16,
                            kind="ExternalInput").ap()
    t_ident = nc.dram_tensor("identity", [P, P], f32,
                             kind="ExternalInput").ap()
    t_W, t_Wal, t_War, t_b = [], [], [], []
    for li, (ind, H, D, _) in enumerate(LAYERS):
        hd = H * D
        t_W.append(nc.dram_tensor(f"W{li}", [ind, hd], f32,
                                  kind="ExternalInput").ap())
        t_Wal.append(nc.dram_tensor(f"Wal{li}", [ind, H], f32,
                                    kind="ExternalInput").ap())
        t_War.append(nc.dram_tensor(f"War{li}", [ind, H], f32,
                                    kind="ExternalInput").ap())
        t_b.append(nc.dram_tensor(f"br{li}", [P, hd], f32,
                                  kind="ExternalInput").ap())
    t_out = nc.dram_tensor("out", [ng * P, OUT_DIM], f32,
                           kind="ExternalOutput").ap()

    with tile.TileContext(nc) as tc:
        with (
            tc.tile_pool(name="const", bufs=1) as cpool,
            tc.tile_pool(name="big", bufs=1) as bigpool,
            tc.tile_pool(name="sb", bufs=3) as sb,
            tc.tile_pool(name="fg", bufs=2) as fgpool,
            tc.tile_pool(name="wp", bufs=2) as wpool,
            tc.tile_pool(name="ps", bufs=3, space="PSUM") as pspool,
            tc.tile_pool(name="psd", bufs=3, space="PSUM") as psdpool,
            tc.tile_pool(name="psA", bufs=2, space="PSUM") as psA,
            tc.tile_pool(name="dram", bufs=1, space="DRAM") as dram,
        ):
            # ---- constants ----
            ident = cpool.tile([P, P], f32)
            nc.sync.dma_start(ident[:], t_ident)
            iota = cpool.tile([P, P], bf16)
            nc.sync.dma_start(iota[:], t_iota)
            idx_lo = cpool.tile([P, SLP // 16], i16)
            nc.sync.dma_start(idx_lo[:], t_idx_lo)
            idx_hi = cpool.tile([P, SHP // 16], i16)
            nc.sync.dma_start(idx_hi[:], t_idx_hi)
            ier_lo = cpool.tile([P, SLP // 16], i16)
            nc.sync.dma_start(ier_lo[:], t_ier_lo)
            ier_hi = cpool.tile([P, SHP // 16], i16)
            nc.sync.dma_start(ier_hi[:], t_ier_hi)
            dr_lo = cpool.tile([P, SLP // P], bf16)
            nc.sync.dma_start(dr_lo[:], t_dr_lo)
            dr_hi = cpool.tile([P, SHP // P], bf16)
            nc.sync.dma_start(dr_hi[:], t_dr_hi)
            xidx = cpool.tile([P, NPCP // 16], i16)
            nc.sync.dma_start(xidx[:], t_x_idx)
            Ws, Wals, Wars, Bs = [], [], [], []
            for li, (ind, H, D, _) in enumerate(LAYERS):
                hd = H * D
                w = cpool.tile([ind, hd], f32, tag=f"W{li}")
                nc.sync.dma_start(w[:], t_W[li])
                Ws.append(w)
                wa = cpool.tile([ind, H], f32, tag=f"Wal{li}")
                nc.sync.dma_start(wa[:], t_Wal[li])
                Wals.append(wa)
                wr = cpool.tile([ind, H], f32, tag=f"War{li}")
                nc.sync.dma_start(wr[:], t_War[li])
                Wars.append(wr)
                bb = cpool.tile([P, hd], f32, tag=f"br{li}")
                nc.sync.dma_start(bb[:], t_b[li])
                Bs.append(bb)

            prev_scratch = None
            for li, (ind, H, D, apply_relu) in enumerate(LAYERS):
                hd = H * D
                # ---------------- phase A ----------------
                x_own = bigpool.tile([P, xj * ind], f32, tag="x_own")
                if li == 0:
                    nc.sync.dma_start(
                        x_own[:].rearrange("p (i d) -> p i d", d=ind),
                        t_feats.rearrange("(i p) d -> p i d", p=P))
                else:
                    _gather(x_own[:].rearrange("p (i d) -> p i d", d=ind),
                            prev_scratch[:], xidx[:], NPCP, ind)
                ROW = ROWS_L[li]
                # combined table rows [feat bf16 | el bf16 | junk]
                tabsb = bigpool.tile([P, xj * ROW], bf16, tag="tabsb")
                nc.vector.memset(tabsb[:], 0.0)
                er_own = sb.tile([P, xj * H], bf16, tag="er_own")
                for i in range(xj):
                    xT_ps = psA.tile([P, P], f32, tag="psA")
                    nc.tensor.transpose(
                        out=xT_ps[:], in_=x_own[:, i * ind:(i + 1) * ind],
                        identity=ident[:])
                    xT = sb.tile([P, ind], f32, tag="xT")
                    nc.any.tensor_copy(xT[:], xT_ps[:, :ind])
                    f_ps = psA.tile([P, hd + 2 * H], f32, tag="psA")
                    nc.tensor.matmul(out=f_ps[:, :hd], lhsT=xT[:],
                                     rhs=Ws[li][:], start=True, stop=True)
                    nc.tensor.matmul(out=f_ps[:, hd:hd + H], lhsT=xT[:],
                                     rhs=Wals[li][:], start=True, stop=True)
                    nc.tensor.matmul(out=f_ps[:, hd + H:hd + 2 * H],
                                     lhsT=xT[:], rhs=Wars[li][:],
                                     start=True, stop=True)
                    nc.any.tensor_copy(
                        tabsb[:, i * ROW:i * ROW + hd], f_ps[:, :hd])
                    nc.any.tensor_copy(
                        tabsb[:, i * ROW + hd:i * ROW + hd + H],
                        f_ps[:, hd:hd + H])
                    nc.any.tensor_copy(er_own[:, i * H:(i + 1) * H],
                                       f_ps[:, hd + H:hd + 2 * H])

                tab_own_d = dram.tile([NPCP, ROW], bf16, tag=f"tab_own{li}")
                nc.sync.dma_start(
                    tab_own_d[:].rearrange("(i p) d -> p i d", p=P),
                    tabsb[:].rearrange("p (i d) -> p i d", d=ROW))
                er_own_d = dram.tile([NPCP, 128], bf16, tag=f"er_own{li}")
                nc.sync.dma_start(
                    er_own_d[:, :H].rearrange("(i p) h -> p i h", p=P),
                    er_own[:].rearrange("p (i h) -> p i h", h=H))

                # ---------------- all-gather ----------------
                tab_full = dram.tile([n_nodes, ROW], bf16,
                                     addr_space="Shared", tag=f"tab_full{li}")
                if n_cores == 1:
                    nc.sync.dma_start(tab_full[:], tab_own_d[:npc, :])
                else:
                    nc.gpsimd.collective_compute(
                        "AllGather", mybir.AluOpType.bypass,
                        replica_groups=[list(range(n_cores))],
                        ins=[tab_own_d[:npc, :]],
                        outs=[tab_full[:]],
                    )

                # ---------------- edge phase ----------------
                if li < 2:
                    scratch = dram.tile([ng * P, hd], f32, tag=f"scratch{li}")
                else:
                    scratch = None

                for k in range(nchunk):
                    g0 = k * CHUNK_GROUPS
                    g1 = g0 + CHUNK_GROUPS
                    lt0, lt1 = int(lo_base[g0]), int(lo_base[g1])
                    ht0, ht1 = int(hi_base[g0]), int(hi_base[g1])
                    zones = []
                    if lt1 > lt0:
                        zones.append(("lo", lt0, lt1, idx_lo, ier_lo, dr_lo,
                                      tab_full[:SPLIT, :]
                                      if n_nodes > SPLIT else tab_full[:]))
                    if ht1 > ht0:
                        zones.append(("hi", ht0, ht1, idx_hi, ier_hi, dr_hi,
                                      tab_full[SPLIT:, :]))
                    ztiles = {}
                    for (zn, tt0, tt1, zidx, zier, zdr, tab_ap) in zones:
                        ntile = tt1 - tt0
                        ni = ntile * P
                        fgt = fgpool.tile([P, ntile * ROW], bf16,
                                          tag=f"fg_{zn}")
                        _gather(fgt[:].rearrange("p (j d) -> p j d", d=ROW),
                                tab_ap, zidx[:, tt0 * 8:tt1 * 8], ni, ROW)
                        erg = fgpool.tile([P, ntile * 128], bf16,
                                          tag=f"er_{zn}")
                        _gather(erg[:].rearrange("p (j d) -> p j d", d=128),
                                er_own_d[:], zier[:, tt0 * 8:tt1 * 8],
                                ni, 128)
                        # w = exp(leaky_relu(el + er))
                        esum = wpool.tile([P, ntile * H], f32,
                                          tag=f"esum_{zn}")
                        nc.vector.tensor_tensor(
                            out=esum[:].rearrange("p (j h) -> p j h", h=H),
                            in0=fgt[:].rearrange("p (j d) -> p j d",
                                                 d=ROW)[:, :, hd:hd + H],
                            in1=erg[:].rearrange("p (j d) -> p j d",
                                                 d=128)[:, :, :H],
                            op=OP.add)
                        esc = wpool.tile([P, ntile * H], f32, tag=f"esc_{zn}")
                        nc.vector.tensor_scalar_mul(esc[:], esum[:],
                                                    NEG_SLOPE)
                        nc.vector.tensor_tensor(out=esum[:], in0=esum[:],
                                                in1=esc[:], op=OP.max)
                        wch = wpool.tile([P, ntile * H], bf16, tag=f"w_{zn}")
                        nc.scalar.activation(wch[:], esum[:], AF.Exp)
                        ztiles[zn] = (fgt, wch, tt0)

                    for g in range(g0, g1):
                        nt_tot = (int(lo_base[g + 1]) - int(lo_base[g]) +
                                  int(hi_base[g + 1]) - int(hi_base[g]))
                        dst_ap = (scratch[g * P:(g + 1) * P, :] if scratch
                                  is not None
                                  else t_out[g * P:(g + 1) * P, :])
                        if nt_tot == 0:
                            # group covers only empty dsts: out = (relu(b))
                            ot = sb.tile([P, hd], f32, tag="ot")
                            if apply_relu:
                                nc.vector.tensor_scalar_max(
                                    ot[:], Bs[li][:, :hd], 0.0)
                            else:
                                nc.vector.tensor_copy(ot[:], Bs[li][:, :hd])
                            nc.sync.dma_start(dst_ap, ot[:])
                            continue
                        ps = pspool.tile([P, hd], f32, tag="ps")
                        psd = psdpool.tile([P, H], f32, tag="psd")
                        first = True
                        done = 0
                        for zn, zb0, zb1, zdr in (
                            ("lo", int(lo_base[g]), int(lo_base[g + 1]),
                             dr_lo),
                            ("hi", int(hi_base[g]), int(hi_base[g + 1]),
                             dr_hi),
                        ):
                            ntg = zb1 - zb0
                            if ntg == 0:
                                continue
                            fgt, wch, tt0 = ztiles[zn]
                            rel = zb0 - tt0
                            oh = sb.tile([P, ntg * P], bf16, tag="oh")
                            nc.vector.tensor_tensor(
                                out=oh[:].rearrange("p (t d) -> p t d", d=P),
                                in0=_midb(iota[:], ntg),
                                in1=zdr[:, zb0:zb1].to_broadcast([P, ntg, P]),
                                op=OP.is_equal)
                            mg = sb.tile([P, ntg * hd], bf16, tag="mg")
                            nc.vector.tensor_tensor(
                                out=mg[:].rearrange("p (t h d) -> p t h d",
                                                    h=H, d=D),
                                in0=fgt[:].rearrange("p (t d) -> p t d",
                                                     d=ROW)[:, rel:rel + ntg,
                                                            :hd]
                                    .rearrange("p t (h d) -> p t h d", d=D),
                                in1=wch[:, rel * H:(rel + ntg) * H]
                                    .rearrange("p (t h) -> p t h", h=H)
                                    .to_broadcast([P, ntg, H, D]),
                                op=OP.mult)
                            for t in range(ntg):
                                done += 1
                                nc.tensor.matmul(
                                    out=ps[:],
                                    lhsT=oh[:, t * P:(t + 1) * P],
                                    rhs=mg[:, t * hd:(t + 1) * hd],
                                    start=first, stop=(done == nt_tot))
                                nc.tensor.matmul(
                                    out=psd[:],
                                    lhsT=oh[:, t * P:(t + 1) * P],
                                    rhs=wch[:, (rel + t) * H:
                                            (rel + t + 1) * H],
                                    start=first, stop=(done == nt_tot))
                                first = False
                        # epilogue: divide, bias, relu
                        den = sb.tile([P, H], f32, tag="den")
                        nc.vector.tensor_scalar_max(den[:], psd[:], 1e-12)
                        rec = sb.tile([P, H], f32, tag="rec")
                        nc.vector.reciprocal(rec[:], den[:])
                        ot = sb.tile([P, hd], f32, tag="ot")
                        nc.vector.tensor_tensor(
                            out=ot[:].rearrange("p (h d) -> p h d", d=D),
                            in0=ps[:].rearrange("p (h d) -> p h d", d=D),
                            in1=rec[:].to_broadcast([P, H, D]),
                            op=OP.mult)
                        nc.vector.tensor_tensor(out=ot[:], in0=ot[:],
                                                in1=Bs[li][:, :hd], op=OP.add)
                        if apply_relu:
                            nc.vector.tensor_scalar_max(ot[:], ot[:], 0.0)
                        nc.sync.dma_start(dst_ap, ot[:])
                prev_scratch = scratch
    nc.compile()
    return nc


# ============================ entry point ====================================

def _meta_key(meta):
    return (meta["ng"], meta["TL"], meta["TH"], meta["SL"], meta["SH"],
            meta["xj"], meta["npc"], meta["n_nodes"], meta["n_cores"])


def _get_compiled(meta):
    key = _meta_key(meta)
    if key not in _cache:
        _cache[key] = _build_program(meta)
    return _cache[key]


def _make_in_maps(inputs, meta, per_core):
    f32 = np.float32
    xj, npc = meta["xj"], meta["npc"]
    n_cores = meta["n_cores"]
    iota_rep = np.tile(np.arange(P, dtype=f32).astype(_BF16), (P, 1))
    ident = np.eye(P, dtype=f32)
    common = {"iota_rep": iota_rep, "identity": ident}
    for li in range(len(LAYERS)):
        ind, H, D, _ = LAYERS[li]
        W = np.asarray(inputs[f"W{li}"], f32)
        al = np.asarray(inputs[f"al{li}"], f32)
        ar = np.asarray(inputs[f"ar{li}"], f32)
        b = np.asarray(inputs[f"b{li}"], f32)
        hd = H * D
        al_flat = np.zeros((hd, H), f32)
        ar_flat = np.zeros((hd, H), f32)
        for h in range(H):
            al_flat[h * D:(h + 1) * D, h] = al[h]
            ar_flat[h * D:(h + 1) * D, h] = ar[h]
        common[f"W{li}"] = W
        common[f"Wal{li}"] = (W @ al_flat).astype(f32)
        common[f"War{li}"] = (W @ ar_flat).astype(f32)
        common[f"br{li}"] = np.tile(b[None, :], (P, 1)).astype(f32)

    feats = np.asarray(inputs["features"], f32)
    in_maps = []
    for c in range(n_cores):
        pc = per_core[c]
        fo = np.zeros((xj * P, 128), f32)
        fo[:npc] = feats[c * npc:(c + 1) * npc]
        in_maps.append({
            **common,
            "features_own": fo,
            "idx_lo": pc["idx_lo"], "idx_hi": pc["idx_hi"],
            "idx_er_lo": pc["idx_er_lo"], "idx_er_hi": pc["idx_er_hi"],
            "dr_lo": pc["dr_lo"], "dr_hi": pc["dr_hi"],
            "x_idx": pc["x_idx"],
        })
    return in_maps


def kernel(**inputs):
    from concourse import bass_utils

    src = np.asarray(inputs["src"]).astype(np.int64)
    dst = np.asarray(inputs["dst"]).astype(np.int64)
    n_nodes = np.asarray(inputs["features"]).shape[0]
    meta, per_core = _preprocess(src, dst, n_nodes, N_CORES)
    nc = _get_compiled(meta)
    in_maps = _make_in_maps(inputs, meta, per_core)
    n_cores = meta["n_cores"]
    res = bass_utils.run_bass_kernel_spmd(
        nc, in_maps, core_ids=list(range(n_cores)),
        trace=bool(last_run_info.get("trace", False)))
    last_run_info["exec_time_ns"] = res.exec_time_ns
    last_run_info["profile_json"] = res.profile_json
    last_run_info["res"] = res

    npc = meta["npc"]
    out = np.empty((n_nodes, OUT_DIM), np.float32)
    for c in range(n_cores):
        rows = per_core[c]["srow"]
        out[c * npc:(c + 1) * npc] = res.results[c]["out"][rows]
    return out
